# revision 33
# baseline (speedup 1.0000x reference)
"""Trainium2 Bass kernel for nn_Mnist_lmdSplineKAN.

Sharding: data-parallel over batch, 8 cores x 128 rows. All params replicated.

fp8(e4m3) weights with feature-weighted error-feedback quantization (x512
pre-scaled, undone in the tanh scale) halve the weight stream to 4.5MB at
358GB/s on the SWDGE queue. Features are built full-width on the DVE as
batched 2x tensor_tensor products+adds (fused Horner forms for the cubic
pieces, f16 +1535.5 floor trick for the interval index); gpsimd assembles
the two single-product planes; the scalar engine does silu and 1-u. Planes
release to the PE in staggered order (j7,j0,1,6,2,5,3,4; last two planes
split by chunk halves). Full-array f16 dummy matmuls right after the
preamble fire the PE HAM clock gate (1.2->2.4GHz) before real work, and a
small re-warm batch sits before the j3 matmul group.

Per-core math (I=784 inputs, H=10 heads, O=64, 8 B-spline basis fns, order 3,
5 uniform intervals on [0,1)):
  v = 5x, tb = v+1535.5 (f16 round -> 1536+floor(v)), u = v - (tb-1536)
  PR2[0]=(1-u)^3  PR2[1]=3u^3-6u^2+4  PR2[2]=3(1-u)^3-6(1-u)^2+4  PR2[3]=u^3
  plane j=t+r gets PR2[r] where t = interval(x); f8 = silu(x)
  y[b,(h,o)] = sum features * Wbig  (f16 x fp8 matmul, psum f32)
  h1 = tanh(y/512); h2 = tanh(h1 @ blockdiag(W1) + b1); logits = <h2,W2> + b2
"""
import sys, types
import numpy as np

B, I, O, H, NB = 1024, 784, 64, 10, 8
NC = 8
BC = B // NC      # 128
CH = 7            # 6 full 128-row chunks + 1 of 16
PLAST = 16
HO = H * O        # 640
D2 = H * 32       # 320
NH = 2
WSCALE = 512.0

GA = (4, 7)       # group A chunk range [4,7)
GB = (0, 4)       # group B chunk range [0,4)


def _install_ntff_hook():
    if "antenv.axon_hooks" in sys.modules:
        return
    try:
        import antenv
        mod = types.ModuleType("antenv.axon_hooks")
        _h = [None]
        mod.set_axon_ntff_profile_hook = lambda h: _h.__setitem__(0, h)
        mod.get_axon_ntff_profile_hook = lambda: _h[0]
        sys.modules["antenv.axon_hooks"] = mod
        antenv.axon_hooks = mod
        from trn_agent_boot.trn_boot import _ntff_profile_via_ctypes
        h = _ntff_profile_via_ctypes("/opt/axon/libaxon_pjrt.so")
        if h is not None:
            mod.set_axon_ntff_profile_hook(h)
    except Exception:
        pass


_CACHE = {}


def _build():
    if "nc" in _CACHE:
        return _CACHE["nc"]
    import concourse.bacc as bacc
    import concourse.bass as bass
    import concourse.tile as tile
    from concourse import mybir
    from contextlib import ExitStack

    f32, f16, f8 = mybir.dt.float32, mybir.dt.float16, mybir.dt.float8e4
    u8 = mybir.dt.uint8
    ALU = mybir.AluOpType
    AF = mybir.ActivationFunctionType

    nc = bacc.Bacc("TRN2", target_bir_lowering=False, debug=False)
    x_d = nc.dram_tensor("x", (128, CH, BC), f32, kind="ExternalInput").ap()
    w_d = nc.dram_tensor("w", (I * (NB + 1) * HO,), f8,
                         kind="ExternalInput").ap()
    w1_d = nc.dram_tensor("w1", (128, 5 * D2 + 128), f16,
                          kind="ExternalInput").ap()
    b1_d = nc.dram_tensor("b1", (1, D2), f16, kind="ExternalInput").ap()
    w2_d = nc.dram_tensor("w2", (128, D2 + H), f32, kind="ExternalInput").ap()
    out_d = nc.dram_tensor("out", (BC, H), f32, kind="ExternalOutput").ap()

    def bcast(ap, n):
        # insert a stride-0 axis of length n in front of the free dims
        return bass.AP(tensor=ap.tensor, offset=ap.offset,
                       ap=[ap.ap[0], [0, n]] + list(ap.ap[1:]))

    with tile.TileContext(nc) as tc, ExitStack() as ctx:
        sb = ctx.enter_context(tc.tile_pool(name="sb", bufs=1))
        ps = ctx.enter_context(tc.tile_pool(name="ps", bufs=1, space="PSUM"))

        # ---- x split across the scalar + gpsimd queues ----
        xt = sb.tile([128, CH, BC], f32, tag="xt")
        nc.sync.dma_start(xt[:, 4:CH, :], x_d[:, 4:CH, :])
        nc.scalar.dma_start(xt[:, 0:4, :], x_d[:, 0:4, :])
        ones = sb.tile([1, 128], f16, tag="ones")
        nc.vector.memset(ones[:], 1.0)
        # gpsimd ucode warm-up (pays any first-use IRAM load early)
        gwu = sb.tile([1, 128], f16, tag="gwu")
        nc.gpsimd.tensor_scalar(gwu[:], ones[:], 1.0, None, op0=ALU.mult)

        # ---- PE clock warm-up: dummy matmuls with no data deps keep the
        #      PE busy from right after the preamble so the HAM ramps the
        #      clock to full rate before the real stream begins ----
        dmy = sb.tile([128, D2], f16, tag="dmy")
        nc.vector.memset(dmy[:], 1.0)
        dmyS = sb.tile([128, 128], f16, tag="dmyS")
        nc.vector.memset(dmyS[:], 1.0)
        ps2 = ps.tile([128, D2], f32, tag="ps2")
        D3 = H * 33
        h2 = sb.tile([128, D3], f32, tag="h2")
        h2v = h2[:].rearrange("p (h d) -> p h d", d=33)
        nc.vector.memset(h2v[:, :, 32], 1.0)

        def dummy_mm(n):
            for _ in range(n):
                nc.tensor.matmul(ps2[:], dmyS[:], dmy[:],
                                 start=True, stop=True)

        # keep-alive: a short matmul whose rhs is a freshly written feature
        # tile; it fires only once that tile is ready, spreading PE activity
        # across the feature phase so the HAM MID window never sees idle
        def keep_alive(ap):
            nc.tensor.matmul(ps2[:, 0:256], dmyS[:], ap,
                             start=True, stop=True)

        dummy_mm(16)

        # ---- weights: piece-major contiguous pieces on the SWDGE queue in
        #      consumption order c4,c5,c6(2),c0..c3 ----
        ROW = (NB + 1) * HO
        wg = [None] * 6
        wg6 = [None, None]

        def w_piece(c):
            t = sb.tile([128, NB + 1, HO], f8, tag=f"wg{c}", name=f"wg{c}")
            src = bass.AP(tensor=w_d.tensor, offset=c * 128 * ROW,
                          ap=[[ROW, 128], [1, ROW]])
            nc.gpsimd.dma_start(t[:], src)
            wg[c] = t

        OFF6 = 6 * 128 * ROW

        def w_piece6(nh):
            t = sb.tile([PLAST, NB + 1, D2], f8, tag=f"wg6{nh}",
                        name=f"wg6{nh}")
            run = (NB + 1) * D2
            src = bass.AP(tensor=w_d.tensor, offset=OFF6 + nh * PLAST * run,
                          ap=[[run, PLAST], [1, run]])
            nc.gpsimd.dma_start(t[:], src)
            wg6[nh] = t

        w_piece(4); w_piece(5); w_piece6(0); w_piece6(1)
        w_piece(0); w_piece(1); w_piece(2); w_piece(3)

        def wslice(c, j, nh):
            if c < 6:
                return wg[c][:, j, nh * D2:(nh + 1) * D2]
            return wg6[nh][:, j, :]

        # ---- tail consts on the scalar queue (after x) ----
        c16 = sb.tile([128, 5 * D2 + 128], f16, tag="c16")
        nc.scalar.dma_start(c16[:], w1_d)
        w1t = c16[:, 0:5 * D2].rearrange("p (k d) -> p k d", d=D2)
        idt = c16[:, 5 * D2:]
        c32 = sb.tile([128, D2 + H], f32, tag="c32")
        nc.scalar.dma_start(c32[:], w2_d)
        w2b = c32[:]
        b1r = sb.tile([1, D2], f16, tag="b1r")
        nc.scalar.dma_start(b1r[:], b1_d)

        # ---- feature tiles (separate per plane: avoids false WAR deps
        #      between plane writes and matmul reads) ----
        Fp = [sb.tile([128, CH, BC], f16, tag=f"F{j}", name=f"F{j}")
              for j in range(NB)]
        fs = sb.tile([128, CH, BC], f16, tag="f8", name="f8")
        nc.scalar.activation(fs[:].rearrange("p c b -> p (c b)"),
                             xt[:].rearrange("p c b -> p (c b)"), AF.Silu)

        vv = sb.tile([128, CH, BC], f16, tag="vv")
        tb = sb.tile([128, CH, BC], f16, tag="tb")
        ti = sb.tile([128, CH, BC], f16, tag="ti")
        uu = sb.tile([128, CH, BC], f16, tag="uu")
        u2 = sb.tile([128, CH, BC], f16, tag="u2")
        ww = sb.tile([128, CH, BC], f16, tag="ww")
        w2 = sb.tile([128, CH, BC], f16, tag="w2")
        zz = sb.tile([128, CH, BC], f16, tag="zz")
        z2 = sb.tile([128, CH, BC], f16, tag="z2")
        t3a = sb.tile([128, CH, BC], f16, tag="t3a")
        t3b = sb.tile([128, CH, BC], f16, tag="t3b")
        MS = sb.tile([128, 5, CH, BC], f16, tag="M")
        PR = sb.tile([128, 4, CH, BC], f16, tag="PR")  # s-order: u3,p2,p1,w3
        tk = sb.tile([128, 4, CH, BC], f16, tag="tk")
        tq = sb.tile([128, 2, CH, BC], f16, tag="tq")

        def fl(t):
            # flat [128, NF] view of a [128, CH, BC] tile (or slice of one)
            return t.rearrange("p c b -> p (c b)")

        def fl2(t, r):
            return t[:, r, :, :].rearrange("p c b -> p (c b)")

        xf = fl(xt)
        vf, tbf, tif, uf = fl(vv), fl(tb), fl(ti), fl(uu)
        u2f, wf, w2f = fl(u2), fl(ww), fl(w2)
        zf, z2f, t3af, t3bf = fl(zz), fl(z2), fl(t3a), fl(t3b)

        # prep chain on DVE; every op is followed by a PE keep-alive so the
        #  HAM never sees an idle window (once throttled, sparse activity
        #  cannot re-fire it)
        def vts(out, a, s1, s2, op0, op1=None):
            kw = {"op0": op0} if op1 is None else {"op0": op0, "op1": op1}
            nc.vector.tensor_scalar(out, a, s1, s2, **kw)

        def vtt(out, a, b, op):
            nc.vector.tensor_tensor(out, a, b, op=op)

        vts(vf, xf, 5.0, None, ALU.mult)
        vts(tbf, vf, 1535.5, None, ALU.add)
        vts(tif, tbf, 1536.0, None, ALU.subtract)
        vtt(uf, vf, tif, ALU.subtract)
        vtt(u2f, uf, uf, ALU.mult)
        nc.scalar.activation(wf, uf, AF.Copy, bias=1.0, scale=-1.0)
        nc.gpsimd.tensor_scalar(zf, u2f, -6.0, 4.0, op0=ALU.mult,
                                op1=ALU.add)
        # masks (DVE TS, cheap)
        for t in range(5):
            vts(fl2(MS, t), tbf, 1536.0 + t, None, ALU.is_equal)
        # pieces in s-order: PR[0]=u^3, PR[1]=3w^3-6w^2+4, PR[2]=3u^3-6u^2+4,
        # PR[3]=w^3   (plane j term t uses s = 3-(j-t))
        vtt(fl2(PR, 0), u2f, uf, ALU.mult)
        vts(t3af, fl2(PR, 0), 3.0, None, ALU.mult)
        vtt(fl2(PR, 2), t3af, zf, ALU.add)
        vtt(w2f, wf, wf, ALU.mult)
        nc.gpsimd.tensor_scalar(z2f, w2f, -6.0, 4.0, op0=ALU.mult,
                                op1=ALU.add)
        vtt(fl2(PR, 3), w2f, wf, ALU.mult)
        vts(t3bf, fl2(PR, 3), 3.0, None, ALU.mult)
        vtt(fl2(PR, 1), t3bf, z2f, ALU.add)
        # gpsimd assembles the two single-product planes in parallel
        nc.gpsimd.tensor_tensor(fl(Fp[7]), fl2(MS, 4), fl2(PR, 0),
                                op=ALU.mult)
        nc.gpsimd.tensor_tensor(fl(Fp[0]), fl2(MS, 0), fl2(PR, 3),
                                op=ALU.mult)

        def r3(t):
            return t.rearrange("p r c b -> p r (c b)")

        # remaining planes on DVE: batched products + tree adds
        def plane(j):
            tlo = max(0, j - 3)
            k = min(4, j) - tlo + 1
            s0 = 3 - min(j, 3)
            out = fl(Fp[j])
            nc.vector.tensor_tensor(
                r3(tk[:, 0:k, :, :]), r3(MS[:, tlo:tlo + k, :, :]),
                r3(PR[:, s0:s0 + k, :, :]), op=ALU.mult)
            if k == 2:
                vtt(out, fl2(tk, 0), fl2(tk, 1), ALU.add)
            elif k == 3:
                nc.vector.tensor_tensor(fl2(tq, 0), fl2(tk, 0), fl2(tk, 1),
                                        op=ALU.add)
                vtt(out, fl2(tq, 0), fl2(tk, 2), ALU.add)
            else:
                nc.vector.tensor_tensor(r3(tq[:]), r3(tk[:, 0:2, :, :]),
                                        r3(tk[:, 2:4, :, :]), op=ALU.add)
                vtt(out, fl2(tq, 0), fl2(tq, 1), ALU.add)

        # split a plane's assembly into chunk ranges (releases the first
        # half to the PE ~2us earlier for the last planes)
        def plane_half(j, c0, c1):
            tlo = max(0, j - 3)
            k = min(4, j) - tlo + 1
            s0 = 3 - min(j, 3)
            w = c1 - c0

            def r3s(t, r0, r1):
                return t[:, r0:r1, c0:c1, :].rearrange("p r c b -> p r (c b)")

            def f2s(t, r):
                return t[:, r, c0:c1, :].rearrange("p c b -> p (c b)")

            out = Fp[j][:, c0:c1, :].rearrange("p c b -> p (c b)")
            nc.vector.tensor_tensor(r3s(tk, 0, k), r3s(MS, tlo, tlo + k),
                                    r3s(PR, s0, s0 + k), op=ALU.mult)
            nc.vector.tensor_tensor(r3s(tq, 0, 2), r3s(tk, 0, 2),
                                    r3s(tk, 2, 4), op=ALU.add)
            nc.vector.tensor_tensor(out, f2s(tq, 0), f2s(tq, 1), op=ALU.add)

        for j in (1, 6, 2, 5):
            plane(j)
        plane_half(3, 4, CH)
        plane_half(3, 0, 4)
        plane_half(4, 4, CH)
        plane_half(4, 0, 4)

        # ---- main matmuls in wavefront order ----
        psum = [ps.tile([128, D2], f32, tag=f"y{nh}", name=f"y{nh}")
                for nh in range(NH)]

        # readiness estimates (us) for emission ordering
        WT = {4: 11.6, 5: 13.7, 6: 13.9, 0: 16.0, 1: 18.1, 2: 20.1, 3: 22.2}
        PT = {NB: 11.8, 7: 17.5, 0: 20.0, 1: 20.5, 6: 22.2, 2: 24.7,
              5: 27.2, 3: 30.5, 4: 33.8}
        FT = {(c, j): PT[j] for c in range(CH) for j in range(NB + 1)}
        for c in range(CH):
            FT[(c, 3)] = 29.0 if c >= 4 else 30.8
            FT[(c, 4)] = 32.6 if c >= 4 else 34.4

        order = sorted(((c, j) for c in range(CH) for j in range(NB + 1)),
                       key=lambda cj: (max(WT[cj[0]], FT[cj]), cj[0]))
        NTOT = CH * (NB + 1)
        # emit in wavefront order; when the PE would idle waiting for the
        # next (c, j) readiness, sprinkle dummy matmuls to keep the HAM
        # clock up (idle > ~3.4us halves the PE clock)
        first3 = min(i for i, cj in enumerate(order) if cj[1] == 3)
        for nmm, (c, j) in enumerate(order):
            if nmm == first3:
                dummy_mm(12)
            if j == NB:
                lhs = (fs[:, c, :] if c < 6 else fs[0:PLAST, c, :])
            else:
                lhs = (Fp[j][:, c, :] if c < 6 else Fp[j][0:PLAST, c, :])
            for nh in range(NH):
                nc.tensor.matmul(
                    psum[nh][:], lhs, wslice(c, j, nh),
                    start=(nmm == 0), stop=(nmm == NTOT - 1))

        # ---- tail: h1 = tanh(y/512), transpose, blockdiag MLP, reduce ----
        h1 = sb.tile([128, HO], f16, tag="h1")
        SEG = [(0, 0, 128), (0, 128, 256), (0, 256, 320), (1, 320, 384),
               (1, 384, 512), (1, 512, 640)]

        def tanh_seg(k):
            nh, s0, s1 = SEG[k]
            nc.scalar.activation(h1[:, s0:s1],
                                 psum[nh][:, s0 - nh * D2:s1 - nh * D2],
                                 AF.Tanh, scale=1.0 / WSCALE)

        h1t = []

        def tr(k):
            pt = ps.tile([128, 128], f16, tag=f"pt{k}", name=f"pt{k}")
            nc.tensor.transpose(pt[:], h1[:, k * 128:(k + 1) * 128], idt)
            st = sb.tile([128, 128], f16, tag=f"h1t{k}", name=f"h1t{k}")
            nc.vector.tensor_copy(st[:], pt[:])
            h1t.append(st)

        tanh_seg(0); tr(0)
        tanh_seg(1); tr(1)
        tanh_seg(2); tanh_seg(3); tr(2)
        tanh_seg(4); tr(3)
        tanh_seg(5); tr(4)

        for k in range(5):
            nc.tensor.matmul(ps2[:], h1t[k][:], w1t[:, k, :],
                             start=(k == 0), stop=False)
        nc.tensor.matmul(ps2[:], ones[:], b1r[:], start=False, stop=True)
        nc.scalar.activation(h2v[:, :, 0:32], ps2[:].rearrange(
            "p (h d) -> p h d", d=32), AF.Tanh)
        prod = sb.tile([128, D3], f32, tag="prod")
        nc.vector.tensor_tensor(prod[:], h2[:], w2b, op=ALU.mult)
        lg = sb.tile([128, H], f32, tag="lg")
        nc.vector.tensor_reduce(lg[:], prod[:].rearrange("p (h d) -> p h d", d=33),
                                axis=mybir.AxisListType.X, op=ALU.add)
        nc.sync.dma_start(out_d, lg[:])

    nc.compile()
    _CACHE["nc"] = nc
    return nc


def _feat_means(xf):
    """mean over batch of the 9 feature planes (6x basis pieces + silu)."""
    v = 5.0 * xf                      # (B, I)
    t = np.floor(v).astype(np.int64)
    u = v - t
    w = 1.0 - u
    PR = np.stack([w ** 3, 3 * u ** 3 - 6 * u ** 2 + 4,
                   3 * w ** 3 - 6 * w ** 2 + 4, u ** 3], -1)  # (B, I, 4)
    F = np.zeros((xf.shape[0], I, NB))
    for r in range(4):
        j = t + r
        np.put_along_axis(F, j[..., None], PR[..., r:r + 1], axis=2)
    silu = xf / (1.0 + np.exp(-xf))
    return np.concatenate([F, silu[..., None]], -1).mean(0)   # (I, 9)


def _quant_feedback(wi, fm):
    """fp8 quantization with feature-weighted error feedback along i.

    wi: (I, 9, H*O) scaled weights; fm: (I, 9) mean feature magnitudes."""
    import ml_dtypes
    f8 = ml_dtypes.float8_e4m3
    q = np.empty_like(wi)
    carry = np.zeros(wi.shape[1:])
    fmc = np.maximum(fm, 1e-6)
    for i in range(I):
        ti = wi[i] + carry
        qt = ti.astype(f8).astype(np.float64)
        q[i] = qt
        ratio = np.clip(fmc[i] / fmc[min(i + 1, I - 1)], 0.0, 2.0)[:, None]
        carry = (ti - qt) * ratio
    return q.astype(f8)


def _prep_inputs(x, coef, scale_base, scale_sp, lmd, W1, b1, W2, b2):
    xf = np.asarray(x, np.float64).reshape(B, I)
    coef = np.asarray(coef, np.float64)
    eff = coef * np.asarray(scale_sp, np.float64)[..., None] \
        * np.asarray(lmd, np.float64)[:, :, None, None] / 6.0
    sbl = np.asarray(scale_base, np.float64) \
        * np.asarray(lmd, np.float64)[:, :, None]
    wbig = np.concatenate([eff, sbl[..., None]], -1) * WSCALE    # (H,I,O,9)
    # -> (I, 9, H, O) with feature-weighted fp8 quantization
    wif = np.ascontiguousarray(wbig.transpose(1, 3, 0, 2))
    wi = _quant_feedback(wif.reshape(I, NB + 1, HO),
                         _feat_means(xf)).reshape(I, NB + 1, H, O)
    pieces = [wi[c * 128:(c + 1) * 128].reshape(-1) for c in range(6)]
    pieces.append(np.ascontiguousarray(wi[768:I, :, 0:5, :]).reshape(-1))
    pieces.append(np.ascontiguousarray(wi[768:I, :, 5:10, :]).reshape(-1))
    wdev = np.concatenate(pieces)

    W1 = np.asarray(W1, np.float64)
    w1bd = np.zeros((HO, D2))
    for h in range(H):
        w1bd[h * O:(h + 1) * O, h * 32:(h + 1) * 32] = W1[h]
    w1dev = np.ascontiguousarray(
        w1bd.reshape(5, 128, D2).transpose(1, 0, 2)).astype(np.float16)
    c16 = np.concatenate([w1dev.reshape(128, 5 * D2),
                          np.eye(128, dtype=np.float16)], 1).astype(np.float16)
    b1c = np.asarray(b1, np.float16).reshape(1, D2).copy()
    w2c = np.concatenate([np.asarray(W2, np.float64).reshape(H, 32),
                          np.asarray(b2, np.float64).reshape(H, 1)], 1)
    c32 = np.ascontiguousarray(np.broadcast_to(
        w2c.reshape(H * 33), (128, H * 33))).astype(np.float32)

    in_maps = []
    for core in range(NC):
        xs = xf[core * BC:(core + 1) * BC].T                     # (784,128)
        xdev = np.zeros((128, CH, BC), np.float32)
        for c in range(CH):
            rows = xs[c * 128:min((c + 1) * 128, I)]
            xdev[0:rows.shape[0], c, :] = rows
        in_maps.append({"x": xdev, "w": wdev, "w1": c16,
                        "b1": b1c, "w2": c32})
    return in_maps


def run(inputs, trace=False, tmpdir=None):
    _install_ntff_hook()
    from concourse.bass_utils import run_bass_kernel_spmd
    nc = _build()
    in_maps = _prep_inputs(**inputs)
    res = run_bass_kernel_spmd(nc, in_maps, core_ids=list(range(NC)),
                               trace=trace, tmpdir=tmpdir)
    out = np.concatenate([r["out"] for r in res.results], 0)
    return out.astype(np.float32), res


def kernel(**inputs):
    out, _ = run(inputs)
    return out


# revision 34
# speedup vs baseline: 1.0725x; 1.0725x over previous
"""Trainium2 Bass kernel for nn_Mnist_lmdSplineKAN.

Sharding: data-parallel over batch, 8 cores x 128 rows. All params replicated.

fp8(e4m3) weights with feature-weighted error-feedback quantization (x512
pre-scaled, undone in the tanh scale) halve the weight stream to 4.5MB at
358GB/s on the SWDGE queue. Features are built full-width on the DVE as
batched 2x tensor_tensor products+adds (fused Horner forms for the cubic
pieces, f16 +1535.5 floor trick for the interval index); gpsimd assembles
the two single-product planes; the scalar engine does silu and 1-u. Planes
release to the PE in staggered order (j7,j0,1,6,2,5,3,4; last two planes
split by chunk halves). Full-array f16 dummy matmuls right after the
preamble fire the PE HAM clock gate (1.2->2.4GHz) before real work, and a
small re-warm batch sits before the j3 matmul group.

Per-core math (I=784 inputs, H=10 heads, O=64, 8 B-spline basis fns, order 3,
5 uniform intervals on [0,1)):
  v = 5x, tb = v+1535.5 (f16 round -> 1536+floor(v)), u = v - (tb-1536)
  PR2[0]=(1-u)^3  PR2[1]=3u^3-6u^2+4  PR2[2]=3(1-u)^3-6(1-u)^2+4  PR2[3]=u^3
  plane j=t+r gets PR2[r] where t = interval(x); f8 = silu(x)
  y[b,(h,o)] = sum features * Wbig  (f16 x fp8 matmul, psum f32)
  h1 = tanh(y/512); h2 = tanh(h1 @ blockdiag(W1) + b1); logits = <h2,W2> + b2
"""
import sys, types
import numpy as np

B, I, O, H, NB = 1024, 784, 64, 10, 8
NC = 8
BC = B // NC      # 128
CH = 7            # 6 full 128-row chunks + 1 of 16
PLAST = 16
HO = H * O        # 640
D2 = H * 32       # 320
NH = 2
WSCALE = 512.0

GA = (4, 7)       # group A chunk range [4,7)
GB = (0, 4)       # group B chunk range [0,4)


def _install_ntff_hook():
    if "antenv.axon_hooks" in sys.modules:
        return
    try:
        import antenv
        mod = types.ModuleType("antenv.axon_hooks")
        _h = [None]
        mod.set_axon_ntff_profile_hook = lambda h: _h.__setitem__(0, h)
        mod.get_axon_ntff_profile_hook = lambda: _h[0]
        sys.modules["antenv.axon_hooks"] = mod
        antenv.axon_hooks = mod
        from trn_agent_boot.trn_boot import _ntff_profile_via_ctypes
        h = _ntff_profile_via_ctypes("/opt/axon/libaxon_pjrt.so")
        if h is not None:
            mod.set_axon_ntff_profile_hook(h)
    except Exception:
        pass


_CACHE = {}


def _build():
    if "nc" in _CACHE:
        return _CACHE["nc"]
    import concourse.bacc as bacc
    import concourse.bass as bass
    import concourse.tile as tile
    from concourse import mybir
    from contextlib import ExitStack

    f32, f16, f8 = mybir.dt.float32, mybir.dt.float16, mybir.dt.float8e4
    u8 = mybir.dt.uint8
    ALU = mybir.AluOpType
    AF = mybir.ActivationFunctionType

    nc = bacc.Bacc("TRN2", target_bir_lowering=False, debug=False)
    x_d = nc.dram_tensor("x", (128, CH, BC), f32, kind="ExternalInput").ap()
    w_d = nc.dram_tensor("w", (I * (NB + 1) * HO,), f8,
                         kind="ExternalInput").ap()
    w1_d = nc.dram_tensor("w1", (128, 5 * D2 + 128), f16,
                          kind="ExternalInput").ap()
    b1_d = nc.dram_tensor("b1", (1, D2), f16, kind="ExternalInput").ap()
    w2_d = nc.dram_tensor("w2", (128, D2 + H), f32, kind="ExternalInput").ap()
    out_d = nc.dram_tensor("out", (BC, H), f32, kind="ExternalOutput").ap()

    def bcast(ap, n):
        # insert a stride-0 axis of length n in front of the free dims
        return bass.AP(tensor=ap.tensor, offset=ap.offset,
                       ap=[ap.ap[0], [0, n]] + list(ap.ap[1:]))

    with tile.TileContext(nc) as tc, ExitStack() as ctx:
        sb = ctx.enter_context(tc.tile_pool(name="sb", bufs=1))
        ps = ctx.enter_context(tc.tile_pool(name="ps", bufs=1, space="PSUM"))

        # ---- x split across the scalar + gpsimd queues ----
        xt = sb.tile([128, CH, BC], f32, tag="xt")
        nc.sync.dma_start(xt[:, 4:CH, :], x_d[:, 4:CH, :])
        nc.scalar.dma_start(xt[:, 0:4, :], x_d[:, 0:4, :])
        ones = sb.tile([1, 128], f16, tag="ones")
        nc.vector.memset(ones[:], 1.0)
        # gpsimd ucode warm-up (pays any first-use IRAM load early)
        gwu = sb.tile([1, 128], f16, tag="gwu")
        nc.gpsimd.tensor_scalar(gwu[:], ones[:], 1.0, None, op0=ALU.mult)

        # ---- PE clock warm-up: dummy matmuls with no data deps keep the
        #      PE busy from right after the preamble so the HAM ramps the
        #      clock to full rate before the real stream begins ----
        dmy = sb.tile([128, D2], f16, tag="dmy")
        nc.vector.memset(dmy[:], 1.0)
        dmyS = sb.tile([128, 128], f16, tag="dmyS")
        nc.vector.memset(dmyS[:], 1.0)
        ps2 = ps.tile([128, D2], f32, tag="ps2")
        D3 = H * 33
        h2 = sb.tile([128, D3], f32, tag="h2")
        h2v = h2[:].rearrange("p (h d) -> p h d", d=33)
        nc.vector.memset(h2v[:, :, 32], 1.0)

        def dummy_mm(n):
            for _ in range(n):
                nc.tensor.matmul(ps2[:], dmyS[:], dmy[:],
                                 start=True, stop=True)

        # keep-alive: a short matmul whose rhs is a freshly written feature
        # tile; it fires only once that tile is ready, spreading PE activity
        # across the feature phase so the HAM MID window never sees idle
        def keep_alive(ap):
            nc.tensor.matmul(ps2[:, 0:256], dmyS[:], ap,
                             start=True, stop=True)

        dummy_mm(16)

        # ---- weights: piece-major contiguous pieces on the SWDGE queue in
        #      consumption order c4,c5,c6(2),c0..c3 ----
        ROW = (NB + 1) * HO
        wg = [None] * 6
        wg6 = [None, None]

        def w_piece(c):
            t = sb.tile([128, NB + 1, HO], f8, tag=f"wg{c}", name=f"wg{c}")
            src = bass.AP(tensor=w_d.tensor, offset=c * 128 * ROW,
                          ap=[[ROW, 128], [1, ROW]])
            nc.gpsimd.dma_start(t[:], src)
            wg[c] = t

        OFF6 = 6 * 128 * ROW

        def w_piece6(nh):
            t = sb.tile([PLAST, NB + 1, D2], f8, tag=f"wg6{nh}",
                        name=f"wg6{nh}")
            run = (NB + 1) * D2
            src = bass.AP(tensor=w_d.tensor, offset=OFF6 + nh * PLAST * run,
                          ap=[[run, PLAST], [1, run]])
            nc.gpsimd.dma_start(t[:], src)
            wg6[nh] = t

        w_piece(4); w_piece(5); w_piece6(0); w_piece6(1)
        w_piece(0); w_piece(1); w_piece(2); w_piece(3)

        def wslice(c, j, nh):
            if c < 6:
                return wg[c][:, j, nh * D2:(nh + 1) * D2]
            return wg6[nh][:, j, :]

        # ---- tail consts on the scalar queue (after x) ----
        c16 = sb.tile([128, 5 * D2 + 128], f16, tag="c16")
        nc.scalar.dma_start(c16[:], w1_d)
        w1t = c16[:, 0:5 * D2].rearrange("p (k d) -> p k d", d=D2)
        idt = c16[:, 5 * D2:]
        c32 = sb.tile([128, D2 + H], f32, tag="c32")
        nc.scalar.dma_start(c32[:], w2_d)
        w2b = c32[:]
        b1r = sb.tile([1, D2], f16, tag="b1r")
        nc.scalar.dma_start(b1r[:], b1_d)

        # ---- feature tiles (separate per plane: avoids false WAR deps
        #      between plane writes and matmul reads) ----
        Fp = [sb.tile([128, CH, BC], f16, tag=f"F{j}", name=f"F{j}")
              for j in range(NB)]
        fs = sb.tile([128, CH, BC], f16, tag="f8", name="f8")
        nc.scalar.activation(fs[:].rearrange("p c b -> p (c b)"),
                             xt[:].rearrange("p c b -> p (c b)"), AF.Silu)

        vv = sb.tile([128, CH, BC], f16, tag="vv")
        tb = sb.tile([128, CH, BC], f16, tag="tb")
        ti = sb.tile([128, CH, BC], f16, tag="ti")
        uu = sb.tile([128, CH, BC], f16, tag="uu")
        u2 = sb.tile([128, CH, BC], f16, tag="u2")
        ww = sb.tile([128, CH, BC], f16, tag="ww")
        w2 = sb.tile([128, CH, BC], f16, tag="w2")
        zz = sb.tile([128, CH, BC], f16, tag="zz")
        z2 = sb.tile([128, CH, BC], f16, tag="z2")
        t3a = sb.tile([128, CH, BC], f16, tag="t3a")
        t3b = sb.tile([128, CH, BC], f16, tag="t3b")
        MS = sb.tile([128, 5, CH, BC], f16, tag="M")
        PR = sb.tile([128, 4, CH, BC], f16, tag="PR")  # s-order: u3,p2,p1,w3
        tk = sb.tile([128, 4, CH, BC], f16, tag="tk")
        tq = sb.tile([128, 2, CH, BC], f16, tag="tq")

        def fl(t):
            # flat [128, NF] view of a [128, CH, BC] tile (or slice of one)
            return t.rearrange("p c b -> p (c b)")

        def fl2(t, r):
            return t[:, r, :, :].rearrange("p c b -> p (c b)")

        xf = fl(xt)
        vf, tbf, tif, uf = fl(vv), fl(tb), fl(ti), fl(uu)
        u2f, wf, w2f = fl(u2), fl(ww), fl(w2)
        zf, z2f, t3af, t3bf = fl(zz), fl(z2), fl(t3a), fl(t3b)

        # prep chain on DVE; every op is followed by a PE keep-alive so the
        #  HAM never sees an idle window (once throttled, sparse activity
        #  cannot re-fire it)
        def vts(out, a, s1, s2, op0, op1=None):
            kw = {"op0": op0} if op1 is None else {"op0": op0, "op1": op1}
            nc.vector.tensor_scalar(out, a, s1, s2, **kw)

        def vtt(out, a, b, op):
            nc.vector.tensor_tensor(out, a, b, op=op)

        vts(vf, xf, 5.0, None, ALU.mult)
        vts(tbf, vf, 1535.5, None, ALU.add)
        vts(tif, tbf, 1536.0, None, ALU.subtract)
        vtt(uf, vf, tif, ALU.subtract)
        nc.scalar.activation(u2f, uf, AF.Square)
        nc.scalar.activation(wf, uf, AF.Copy, bias=1.0, scale=-1.0)
        nc.scalar.activation(w2f, uf, AF.Square, bias=1.0, scale=-1.0)
        nc.gpsimd.tensor_scalar(zf, u2f, -6.0, 4.0, op0=ALU.mult,
                                op1=ALU.add)
        # masks (DVE TS, cheap)
        for t in range(5):
            vts(fl2(MS, t), tbf, 1536.0 + t, None, ALU.is_equal)
        # pieces in s-order: PR[0]=u^3, PR[1]=3w^3-6w^2+4, PR[2]=3u^3-6u^2+4,
        # PR[3]=w^3   (plane j term t uses s = 3-(j-t))
        vtt(fl2(PR, 0), u2f, uf, ALU.mult)
        vts(t3af, fl2(PR, 0), 3.0, None, ALU.mult)
        vtt(fl2(PR, 2), t3af, zf, ALU.add)
        nc.gpsimd.tensor_scalar(z2f, w2f, -6.0, 4.0, op0=ALU.mult,
                                op1=ALU.add)
        vtt(fl2(PR, 3), w2f, wf, ALU.mult)
        vts(t3bf, fl2(PR, 3), 3.0, None, ALU.mult)
        vtt(fl2(PR, 1), t3bf, z2f, ALU.add)
        # gpsimd assembles the two single-product planes in parallel
        nc.gpsimd.tensor_tensor(fl(Fp[7]), fl2(MS, 4), fl2(PR, 0),
                                op=ALU.mult)
        nc.gpsimd.tensor_tensor(fl(Fp[0]), fl2(MS, 0), fl2(PR, 3),
                                op=ALU.mult)

        def r3(t):
            return t.rearrange("p r c b -> p r (c b)")

        # remaining planes on DVE: batched products + tree adds
        def plane(j):
            tlo = max(0, j - 3)
            k = min(4, j) - tlo + 1
            s0 = 3 - min(j, 3)
            out = fl(Fp[j])
            nc.vector.tensor_tensor(
                r3(tk[:, 0:k, :, :]), r3(MS[:, tlo:tlo + k, :, :]),
                r3(PR[:, s0:s0 + k, :, :]), op=ALU.mult)
            if k == 2:
                vtt(out, fl2(tk, 0), fl2(tk, 1), ALU.add)
            elif k == 3:
                nc.vector.tensor_tensor(fl2(tq, 0), fl2(tk, 0), fl2(tk, 1),
                                        op=ALU.add)
                vtt(out, fl2(tq, 0), fl2(tk, 2), ALU.add)
            else:
                nc.vector.tensor_tensor(r3(tq[:]), r3(tk[:, 0:2, :, :]),
                                        r3(tk[:, 2:4, :, :]), op=ALU.add)
                vtt(out, fl2(tq, 0), fl2(tq, 1), ALU.add)

        # split a plane's assembly into chunk ranges (releases the first
        # half to the PE ~2us earlier for the last planes)
        def plane_half(j, c0, c1):
            tlo = max(0, j - 3)
            k = min(4, j) - tlo + 1
            s0 = 3 - min(j, 3)
            w = c1 - c0

            def r3s(t, r0, r1):
                return t[:, r0:r1, c0:c1, :].rearrange("p r c b -> p r (c b)")

            def f2s(t, r):
                return t[:, r, c0:c1, :].rearrange("p c b -> p (c b)")

            out = Fp[j][:, c0:c1, :].rearrange("p c b -> p (c b)")
            nc.vector.tensor_tensor(r3s(tk, 0, k), r3s(MS, tlo, tlo + k),
                                    r3s(PR, s0, s0 + k), op=ALU.mult)
            nc.vector.tensor_tensor(r3s(tq, 0, 2), r3s(tk, 0, 2),
                                    r3s(tk, 2, 4), op=ALU.add)
            nc.vector.tensor_tensor(out, f2s(tq, 0), f2s(tq, 1), op=ALU.add)

        for j in (1, 6, 2, 5):
            plane(j)
        plane_half(3, 4, CH)
        plane_half(3, 0, 4)
        plane_half(4, 4, CH)
        plane_half(4, 0, 4)

        # ---- main matmuls in wavefront order ----
        psum = [ps.tile([128, D2], f32, tag=f"y{nh}", name=f"y{nh}")
                for nh in range(NH)]

        # readiness estimates (us) for emission ordering
        WT = {4: 11.6, 5: 13.7, 6: 13.9, 0: 16.0, 1: 18.1, 2: 20.1, 3: 22.2}
        PT = {NB: 11.8, 7: 17.5, 0: 20.0, 1: 20.5, 6: 22.2, 2: 24.7,
              5: 27.2, 3: 30.5, 4: 33.8}
        FT = {(c, j): PT[j] for c in range(CH) for j in range(NB + 1)}
        for c in range(CH):
            FT[(c, 3)] = 29.0 if c >= 4 else 30.8
            FT[(c, 4)] = 32.6 if c >= 4 else 34.4

        order = sorted(((c, j) for c in range(CH) for j in range(NB + 1)),
                       key=lambda cj: (max(WT[cj[0]], FT[cj]), cj[0]))
        NTOT = CH * (NB + 1)
        # emit in wavefront order; when the PE would idle waiting for the
        # next (c, j) readiness, sprinkle dummy matmuls to keep the HAM
        # clock up (idle > ~3.4us halves the PE clock)
        first3 = min(i for i, cj in enumerate(order) if cj[1] == 3)
        for nmm, (c, j) in enumerate(order):
            if nmm == first3:
                dummy_mm(12)
            if j == NB:
                lhs = (fs[:, c, :] if c < 6 else fs[0:PLAST, c, :])
            else:
                lhs = (Fp[j][:, c, :] if c < 6 else Fp[j][0:PLAST, c, :])
            for nh in range(NH):
                nc.tensor.matmul(
                    psum[nh][:], lhs, wslice(c, j, nh),
                    start=(nmm == 0), stop=(nmm == NTOT - 1))

        # ---- tail: h1 = tanh(y/512), transpose, blockdiag MLP, reduce ----
        h1 = sb.tile([128, HO], f16, tag="h1")
        SEG = [(0, 0, 128), (0, 128, 256), (0, 256, 320), (1, 320, 384),
               (1, 384, 512), (1, 512, 640)]

        def tanh_seg(k):
            nh, s0, s1 = SEG[k]
            nc.scalar.activation(h1[:, s0:s1],
                                 psum[nh][:, s0 - nh * D2:s1 - nh * D2],
                                 AF.Tanh, scale=1.0 / WSCALE)

        h1t = []

        def tr(k):
            pt = ps.tile([128, 128], f16, tag=f"pt{k}", name=f"pt{k}")
            nc.tensor.transpose(pt[:], h1[:, k * 128:(k + 1) * 128], idt)
            st = sb.tile([128, 128], f16, tag=f"h1t{k}", name=f"h1t{k}")
            nc.vector.tensor_copy(st[:], pt[:])
            h1t.append(st)

        tanh_seg(0); tr(0)
        tanh_seg(1); tr(1)
        tanh_seg(2); tanh_seg(3); tr(2)
        tanh_seg(4); tr(3)
        tanh_seg(5); tr(4)

        for k in range(5):
            nc.tensor.matmul(ps2[:], h1t[k][:], w1t[:, k, :],
                             start=(k == 0), stop=False)
        nc.tensor.matmul(ps2[:], ones[:], b1r[:], start=False, stop=True)
        nc.scalar.activation(h2v[:, :, 0:32], ps2[:].rearrange(
            "p (h d) -> p h d", d=32), AF.Tanh)
        prod = sb.tile([128, D3], f32, tag="prod")
        nc.vector.tensor_tensor(prod[:], h2[:], w2b, op=ALU.mult)
        lg = sb.tile([128, H], f32, tag="lg")
        nc.vector.tensor_reduce(lg[:], prod[:].rearrange("p (h d) -> p h d", d=33),
                                axis=mybir.AxisListType.X, op=ALU.add)
        nc.sync.dma_start(out_d, lg[:])

    nc.compile()
    _CACHE["nc"] = nc
    return nc


def _feat_means(xf):
    """mean over batch of the 9 feature planes (6x basis pieces + silu)."""
    v = 5.0 * xf                      # (B, I)
    t = np.floor(v).astype(np.int64)
    u = v - t
    w = 1.0 - u
    PR = np.stack([w ** 3, 3 * u ** 3 - 6 * u ** 2 + 4,
                   3 * w ** 3 - 6 * w ** 2 + 4, u ** 3], -1)  # (B, I, 4)
    F = np.zeros((xf.shape[0], I, NB))
    for r in range(4):
        j = t + r
        np.put_along_axis(F, j[..., None], PR[..., r:r + 1], axis=2)
    silu = xf / (1.0 + np.exp(-xf))
    return np.concatenate([F, silu[..., None]], -1).mean(0)   # (I, 9)


def _quant_feedback(wi, fm):
    """fp8 quantization with feature-weighted error feedback along i.

    wi: (I, 9, H*O) scaled weights; fm: (I, 9) mean feature magnitudes."""
    import ml_dtypes
    f8 = ml_dtypes.float8_e4m3
    q = np.empty_like(wi)
    carry = np.zeros(wi.shape[1:])
    fmc = np.maximum(fm, 1e-6)
    for i in range(I):
        ti = wi[i] + carry
        qt = ti.astype(f8).astype(np.float64)
        q[i] = qt
        ratio = np.clip(fmc[i] / fmc[min(i + 1, I - 1)], 0.0, 2.0)[:, None]
        carry = (ti - qt) * ratio
    return q.astype(f8)


def _prep_inputs(x, coef, scale_base, scale_sp, lmd, W1, b1, W2, b2):
    xf = np.asarray(x, np.float64).reshape(B, I)
    coef = np.asarray(coef, np.float64)
    eff = coef * np.asarray(scale_sp, np.float64)[..., None] \
        * np.asarray(lmd, np.float64)[:, :, None, None] / 6.0
    sbl = np.asarray(scale_base, np.float64) \
        * np.asarray(lmd, np.float64)[:, :, None]
    wbig = np.concatenate([eff, sbl[..., None]], -1) * WSCALE    # (H,I,O,9)
    # -> (I, 9, H, O) with feature-weighted fp8 quantization
    wif = np.ascontiguousarray(wbig.transpose(1, 3, 0, 2))
    wi = _quant_feedback(wif.reshape(I, NB + 1, HO),
                         _feat_means(xf)).reshape(I, NB + 1, H, O)
    pieces = [wi[c * 128:(c + 1) * 128].reshape(-1) for c in range(6)]
    pieces.append(np.ascontiguousarray(wi[768:I, :, 0:5, :]).reshape(-1))
    pieces.append(np.ascontiguousarray(wi[768:I, :, 5:10, :]).reshape(-1))
    wdev = np.concatenate(pieces)

    W1 = np.asarray(W1, np.float64)
    w1bd = np.zeros((HO, D2))
    for h in range(H):
        w1bd[h * O:(h + 1) * O, h * 32:(h + 1) * 32] = W1[h]
    w1dev = np.ascontiguousarray(
        w1bd.reshape(5, 128, D2).transpose(1, 0, 2)).astype(np.float16)
    c16 = np.concatenate([w1dev.reshape(128, 5 * D2),
                          np.eye(128, dtype=np.float16)], 1).astype(np.float16)
    b1c = np.asarray(b1, np.float16).reshape(1, D2).copy()
    w2c = np.concatenate([np.asarray(W2, np.float64).reshape(H, 32),
                          np.asarray(b2, np.float64).reshape(H, 1)], 1)
    c32 = np.ascontiguousarray(np.broadcast_to(
        w2c.reshape(H * 33), (128, H * 33))).astype(np.float32)

    in_maps = []
    for core in range(NC):
        xs = xf[core * BC:(core + 1) * BC].T                     # (784,128)
        xdev = np.zeros((128, CH, BC), np.float32)
        for c in range(CH):
            rows = xs[c * 128:min((c + 1) * 128, I)]
            xdev[0:rows.shape[0], c, :] = rows
        in_maps.append({"x": xdev, "w": wdev, "w1": c16,
                        "b1": b1c, "w2": c32})
    return in_maps


def run(inputs, trace=False, tmpdir=None):
    _install_ntff_hook()
    from concourse.bass_utils import run_bass_kernel_spmd
    nc = _build()
    in_maps = _prep_inputs(**inputs)
    res = run_bass_kernel_spmd(nc, in_maps, core_ids=list(range(NC)),
                               trace=trace, tmpdir=tmpdir)
    out = np.concatenate([r["out"] for r in res.results], 0)
    return out.astype(np.float32), res


def kernel(**inputs):
    out, _ = run(inputs)
    return out
